# revision 11
# baseline (speedup 1.0000x reference)
"""Trainium2 Bass kernel for MixGRU: y = ((GRU_last(x @ Wmix.T)) @ Whead.T + bhead) @ Wmix.

Data-parallel over batch across 8 NeuronCores (32 batch elements per core).
All recurrent state kept transposed ([HID, B] tiles) so the sequential GRU
scan runs on cheap 96-partition ops. Input-side gate projections are
precomputed (fp32) in a pipelined fashion and injected into the scan's PSUM
banks via an identity-matmul (issued one step ahead, start=True); the
hidden-state matmuls run in bf16 (fp32 PSUM accumulate) with b_hh_n folded
in through hi/lo-split ones-rows of the state tile.
"""

import numpy as np

import concourse.bass as bass
import concourse.mybir as mybir
from concourse import bacc, tile
from concourse.bass_utils import run_bass_kernel_spmd

F32 = mybir.dt.float32
F16 = mybir.dt.float16
AFT = mybir.ActivationFunctionType
OP = mybir.AluOpType

B, T, D = 256, 512, 512
MIX, HID = 32, 96
NCORES = 8
BS = B // NCORES          # 32 batch per core
BLK = 16                  # scan steps per pipeline block
COLS = BLK * BS           # 512 columns per block
KH = HID + 2              # state rows + two ones-rows (bias hi/lo)

TRACE = False
LAST_EXEC_NS = None
_CACHE = {}


def _seq(*fs):
    def f():
        for g in fs:
            g()
    return f


def build(t_total=T):
    nblk = t_total // BLK
    nc = bacc.Bacc("TRN2", target_bir_lowering=False, debug=False)

    xT = nc.dram_tensor("xT", [D, t_total * BS], F32, kind="ExternalInput")
    WzT = nc.dram_tensor("WzT", [128, 4, MIX], F32, kind="ExternalInput")
    Wih = nc.dram_tensor("Wih", [MIX + 1, 3 * HID], F32, kind="ExternalInput")
    # bf16 stationaries for the scan: gates r, u, -u, n ([KH, 4*HID]);
    # rows HID:KH are zeros except the n-gate, which carries b_hh_n hi/lo.
    Whh = nc.dram_tensor("Whh", [KH, 4 * HID], F16, kind="ExternalInput")
    I96 = nc.dram_tensor("I96", [HID, HID], F16, kind="ExternalInput")
    WheadT = nc.dram_tensor("WheadT", [HID, MIX], F32, kind="ExternalInput")
    bhead = nc.dram_tensor("bhead", [MIX, 1], F32, kind="ExternalInput")
    Wmix = nc.dram_tensor("Wmix", [MIX, D], F32, kind="ExternalInput")
    yT = nc.dram_tensor("yT", [D, BS], F32, kind="ExternalOutput")

    with tile.TileContext(nc) as tc:
        with (
            tc.tile_pool(name="wts", bufs=1) as wts,
            tc.tile_pool(name="xp", bufs=9) as xp,
            tc.tile_pool(name="zp", bufs=2) as zp,
            tc.tile_pool(name="gbp", bufs=3) as gbp,
            tc.tile_pool(name="gnp", bufs=3) as gnp,
            tc.tile_pool(name="hp", bufs=3) as hp,
            tc.tile_pool(name="gate", bufs=3) as gate,
            tc.tile_pool(name="outp", bufs=2) as outp,
            tc.tile_pool(name="zps", bufs=1, space="PSUM") as zps,
            tc.tile_pool(name="gxps", bufs=3, space="PSUM") as gxps,
            tc.tile_pool(name="ps1", bufs=2, space="PSUM") as ps1p,
            tc.tile_pool(name="ps2", bufs=2, space="PSUM") as ps2p,
        ):
            # ---- persistent weights in SBUF ----
            wz = wts.tile([128, 4, MIX], F32, tag="wz")
            nc.sync.dma_start(wz[:], WzT[:])
            wih = wts.tile([MIX + 1, 3 * HID], F32, tag="wih")
            nc.sync.dma_start(wih[:], Wih[:])
            whh = wts.tile([KH, 4 * HID], F16, tag="whh")
            nc.sync.dma_start(whh[:], Whh[:])
            i96 = wts.tile([HID, HID], F16, tag="i96")
            nc.sync.dma_start(i96[:], I96[:])
            whd = wts.tile([HID, MIX], F32, tag="whd")
            nc.sync.dma_start(whd[:], WheadT[:])
            bhd = wts.tile([MIX, 1], F32, tag="bhd")
            nc.sync.dma_start(bhd[:], bhead[:])
            wmx = wts.tile([MIX, D], F32, tag="wmx")
            nc.sync.dma_start(wmx[:], Wmix[:])

            # ---- ACT table warmup (sigmoid/tanh share one table set) ----
            scr = gate.tile([HID, BS], F32, tag="scr")
            nc.gpsimd.memset(scr[:], 0.0)
            nc.scalar.activation(scr[:], scr[:], AFT.Sigmoid)
            nc.scalar.activation(scr[:], scr[:], AFT.Tanh)

            # ---- initial hidden state ----
            h = hp.tile([KH, BS], F16)
            nc.gpsimd.memset(h[0:HID, :], 0.0)
            nc.gpsimd.memset(h[HID:KH, :], 1.0)

            def dma_block(j):
                xts = []
                for k in range(4):
                    xt = xp.tile([128, COLS], F32)
                    nc.sync.dma_start(
                        xt[:], xT[k * 128:(k + 1) * 128, j * COLS:(j + 1) * COLS]
                    )
                    xts.append(xt)
                return xts

            def make_chunks(j, xts):
                """Precompute block j as a list of small closures, each sized
                to hide inside one scan step's PE/DVE idle window.

                gb[:, i, :] holds fp16 (gxb_r | gxb_u | -gxb_u) for step i;
                gn holds fp32 gx_n (t-major, 32 batch cols per step)."""
                HC = COLS // 2  # 256-column halves
                ztile = zp.tile([MIX + 1, COLS], F32)
                zpsum = zps.tile([MIX, COLS], F32)
                gb = gbp.tile([HID, BLK, 3 * BS], F16)
                gn = gnp.tile([HID, COLS], F32)
                gps_half = {}
                pieces = []

                def zmm(k, hh):
                    def f():
                        nc.tensor.matmul(
                            zpsum[:, hh * HC:(hh + 1) * HC],
                            wz[:, k, :], xts[k][:, hh * HC:(hh + 1) * HC],
                            start=(k == 0), stop=(k == 3),
                        )
                    return f

                def zcopy(hh):
                    def f():
                        nc.vector.tensor_copy(
                            ztile[0:MIX, hh * HC:(hh + 1) * HC],
                            zpsum[:, hh * HC:(hh + 1) * HC],
                        )
                        if hh == 0:
                            nc.gpsimd.memset(ztile[MIX:MIX + 1, :], 1.0)
                    return f

                def gxmm(gi, hh):
                    def f():
                        gps = gxps.tile([HID, HC], F32)
                        gps_half[(gi, hh)] = gps
                        nc.tensor.matmul(
                            gps[:], wih[:, gi * HID:(gi + 1) * HID],
                            ztile[:, hh * HC:(hh + 1) * HC],
                            start=True, stop=True,
                        )
                    return f

                def gcopy(gi, hh):
                    def f():
                        gps = gps_half.pop((gi, hh))
                        src = gps[:].rearrange("p (t b) -> p t b", b=BS)
                        trng = slice(hh * (BLK // 2), (hh + 1) * (BLK // 2))
                        if gi < 2:
                            nc.vector.tensor_copy(
                                gb[:, trng, gi * BS:(gi + 1) * BS], src
                            )
                            if gi == 1:  # negated copy for the (1-u) column
                                nc.vector.tensor_scalar(
                                    gb[:, trng, 2 * BS:3 * BS], src,
                                    -1.0, None, op0=OP.mult,
                                )
                        else:
                            nc.vector.tensor_copy(
                                gn[:, hh * HC:(hh + 1) * HC], gps[:]
                            )
                    return f

                for k in range(4):
                    pieces.append(zmm(k, 0))
                for k in range(4):
                    pieces.append(zmm(k, 1))
                pieces[3] = _seq(pieces[3], zcopy(0))
                pieces[7] = _seq(pieces[7], zcopy(1))
                # gx matmuls interleaved with their evacuation copies
                pieces.append(gxmm(0, 0))            # p8
                pieces.append(_seq(gxmm(1, 0), gcopy(0, 0)))   # p9
                pieces.append(_seq(gxmm(2, 0), gcopy(1, 0)))   # p10
                pieces.append(_seq(gxmm(0, 1), gcopy(2, 0)))   # p11
                pieces.append(_seq(gxmm(1, 1), gcopy(0, 1)))   # p12
                pieces.append(_seq(gxmm(2, 1), gcopy(1, 1)))   # p13
                pieces.append(gcopy(2, 1))           # p14
                return gb, gn, pieces

            def imm(gb, i):
                """Inject precomputed gate inputs for step i into a fresh
                PSUM bank (start=True) — issued one step ahead."""
                ps1 = ps1p.tile([HID, 3 * BS], F32, tag="ps1")
                nc.tensor.matmul(ps1[:], i96[:], gb[:, i, :],
                                 start=True, stop=False)
                return ps1

            def scan_step(h, ps1, gn, i):
                nc.tensor.matmul(ps1[:, 0:BS], whh[:, 0:HID], h[:],
                                 start=False, stop=False)
                nc.tensor.matmul(ps1[:, BS:2 * BS], whh[:, HID:2 * HID], h[:],
                                 start=False, stop=False)
                nc.tensor.matmul(ps1[:, 2 * BS:3 * BS], whh[:, 2 * HID:3 * HID],
                                 h[:], start=False, stop=True)
                ps2 = ps2p.tile([HID, BS], F32, tag="ps2")
                nc.tensor.matmul(ps2[:, 0:BS], whh[:, 3 * HID:4 * HID], h[:],
                                 start=True, stop=True)

                r = gate.tile([HID, BS], F32, tag="r")
                nc.scalar.activation(r[:], ps1[:, 0:BS], AFT.Sigmoid)
                uu = gate.tile([HID, 2 * BS], F32, tag="uu")
                nc.scalar.activation(uu[:], ps1[:, BS:3 * BS], AFT.Sigmoid)

                tn = gate.tile([HID, BS], F32, tag="tn")
                nc.vector.tensor_tensor(tn[:], ps2[:, 0:BS], r[:], op=OP.mult)
                tn2 = gate.tile([HID, BS], F32, tag="tn2")
                nc.vector.tensor_tensor(
                    tn2[:], tn[:], gn[:, i * BS:(i + 1) * BS], op=OP.add,
                )
                nn = gate.tile([HID, BS], F32, tag="nn")
                nc.scalar.activation(nn[:], tn2[:], AFT.Tanh)

                uh = gate.tile([HID, BS], F32, tag="uh")
                nc.vector.tensor_tensor(uh[:], uu[:, 0:BS], h[0:HID, :],
                                        op=OP.mult)
                h2 = hp.tile([KH, BS], F16)
                nc.gpsimd.memset(h2[HID:KH, :], 1.0)
                nc.vector.tensor_tensor(h2[0:HID, :], uu[:, BS:2 * BS], nn[:],
                                        op=OP.mult)
                nc.vector.tensor_tensor(h2[0:HID, :], h2[0:HID, :], uh[:],
                                        op=OP.add)
                return h2

            # ---- pipelined precompute + scan ----
            # x-DMAs issued two blocks ahead of their matmuls; precompute
            # pieces for block j+2 drip one-per-step through block j.
            blocks = {}
            for j in range(min(3, nblk)):
                xts = dma_block(j)
                if j < 2:
                    gbj, gnj, pieces = make_chunks(j, xts)
                    for p in pieces:
                        p()
                    blocks[j] = (gbj, gnj, xts)
                else:
                    blocks[j] = (None, None, xts)

            ps1 = imm(blocks[0][0], 0)
            for j in range(nblk):
                if j + 3 < nblk:
                    blocks[j + 3] = (None, None, dma_block(j + 3))
                pend = []
                if j + 2 < nblk:
                    gbj, gnj, pieces = make_chunks(j + 2, blocks[j + 2][2])
                    blocks[j + 2] = (gbj, gnj, None)
                    pend = pieces
                cur_gb, cur_gn = blocks[j][0], blocks[j][1]
                for i in range(BLK):
                    h2 = scan_step(h, ps1, cur_gn, i)
                    if i < len(pend):
                        pend[i]()
                    # inject next step's gate inputs while this chain runs
                    last = (j == nblk - 1) and (i == BLK - 1)
                    if not last:
                        if i == BLK - 1:
                            ps1 = imm(blocks[j + 1][0], 0)
                        else:
                            ps1 = imm(cur_gb, i + 1)
                    h = h2
                blocks.pop(j)

            # ---- head: z_next = Whead @ h + bhead ; y.T = Wmix.T @ z_next ----
            hf = gate.tile([HID, BS], F32, tag="hf")
            nc.vector.tensor_copy(hf[:], h[0:HID, :])
            znps = ps1p.tile([MIX, BS], F32, tag="ps1")
            nc.tensor.matmul(znps[:], whd[:], hf[:], start=True, stop=True)
            zn = gate.tile([MIX, BS], F32, tag="zn")
            nc.vector.tensor_scalar(zn[:], znps[:], bhd[:], None, op0=OP.add)
            for k in range(4):
                yps = ps2p.tile([128, BS], F32, tag="ps2")
                nc.tensor.matmul(yps[:], wmx[:, k * 128:(k + 1) * 128], zn[:],
                                 start=True, stop=True)
                yt = outp.tile([128, BS], F32)
                nc.vector.tensor_copy(yt[:], yps[:])
                nc.sync.dma_start(yT[k * 128:(k + 1) * 128, :], yt[:])

    nc.compile()
    return nc


def _f16(a):
    return np.asarray(a, np.float32).astype(np.float16)


def prep_weights(W_mix, W_ih, W_hh, b_ih, b_hh, W_head, b_head):
    W_mix = np.asarray(W_mix, np.float32)
    W_ih = np.asarray(W_ih, np.float32)
    W_hh = np.asarray(W_hh, np.float32)
    b_ih = np.asarray(b_ih, np.float32)
    b_hh = np.asarray(b_hh, np.float32)
    W_head = np.asarray(W_head, np.float32)
    b_head = np.asarray(b_head, np.float32)

    # WzT[p, k, m] = W_mix[m, 128k + p]
    WzT = np.ascontiguousarray(
        W_mix.T.reshape(4, 128, MIX).transpose(1, 0, 2)
    )
    # Wih_hat: [MIX+1, 3H]; per gate columns = [W_ih_g.T ; fused bias]
    gates_b = [
        b_ih[0:HID] + b_hh[0:HID],
        b_ih[HID:2 * HID] + b_hh[HID:2 * HID],
        b_ih[2 * HID:3 * HID],
    ]
    Wih_hat = np.zeros((MIX + 1, 3 * HID), np.float32)
    for g in range(3):
        Wih_hat[0:MIX, g * HID:(g + 1) * HID] = W_ih[g * HID:(g + 1) * HID].T
        Wih_hat[MIX, g * HID:(g + 1) * HID] = gates_b[g]

    # bf16 scan stationaries [KH, 4H]: r, u, -u, n; n carries b_hh_n hi/lo.
    Whh_hat = np.zeros((KH, 4 * HID), np.float32)
    Wr, Wu, Wn = (W_hh[g * HID:(g + 1) * HID] for g in range(3))
    Whh_hat[0:HID, 0:HID] = Wr.T
    Whh_hat[0:HID, HID:2 * HID] = Wu.T
    Whh_hat[0:HID, 2 * HID:3 * HID] = -Wu.T
    Whh_hat[0:HID, 3 * HID:4 * HID] = Wn.T
    bn = b_hh[2 * HID:3 * HID]
    bn_hi = bn.astype(np.float16).astype(np.float32)
    Whh_hat[HID, 3 * HID:4 * HID] = bn_hi
    Whh_hat[HID + 1, 3 * HID:4 * HID] = bn - bn_hi
    return {
        "WzT": WzT,
        "Wih": Wih_hat,
        "Whh": _f16(Whh_hat),
        "I96": _f16(np.eye(HID, dtype=np.float32)),
        "WheadT": np.ascontiguousarray(W_head.T),
        "bhead": np.ascontiguousarray(b_head[:, None]),
        "Wmix": W_mix,
    }


def kernel(x, W_mix, W_ih, W_hh, b_ih, b_hh, W_head, b_head):
    global LAST_EXEC_NS
    if "nc" not in _CACHE:
        _CACHE["nc"] = build(T)
    nc = _CACHE["nc"]

    wmap = prep_weights(W_mix, W_ih, W_hh, b_ih, b_hh, W_head, b_head)
    x = np.asarray(x, np.float32)
    in_maps = []
    for c in range(NCORES):
        xc = x[c * BS:(c + 1) * BS]                       # [BS, T, D]
        xTc = np.ascontiguousarray(xc.transpose(2, 1, 0)).reshape(D, T * BS)
        in_maps.append({"xT": xTc, **wmap})

    res = run_bass_kernel_spmd(
        nc, in_maps, core_ids=list(range(NCORES)), trace=TRACE
    )
    LAST_EXEC_NS = res.exec_time_ns
    y = np.empty((B, D), np.float32)
    for c in range(NCORES):
        y[c * BS:(c + 1) * BS] = res.results[c]["yT"].T
    return y


# revision 12
# speedup vs baseline: 1.1781x; 1.1781x over previous
"""Trainium2 Bass kernel for MixGRU: y = ((GRU_last(x @ Wmix.T)) @ Whead.T + bhead) @ Wmix.

Data-parallel over batch across 8 NeuronCores (32 batch elements per core).
All recurrent state kept transposed ([HID, B] tiles) so the sequential GRU
scan runs on cheap 96-partition ops. Input-side gate projections are
precomputed (fp32) in a pipelined fashion and injected into the scan's PSUM
banks via an identity-matmul (issued one step ahead, start=True); the
hidden-state matmuls run in bf16 (fp32 PSUM accumulate) with b_hh_n folded
in through hi/lo-split ones-rows of the state tile.
"""

import numpy as np

import concourse.bass as bass
import concourse.mybir as mybir
from concourse import bacc, tile
from concourse.tile_rust import add_dep_helper
from concourse.bass_utils import run_bass_kernel_spmd

F32 = mybir.dt.float32
F16 = mybir.dt.float16
AFT = mybir.ActivationFunctionType
OP = mybir.AluOpType

B, T, D = 256, 512, 512
MIX, HID = 32, 96
NCORES = 8
BS = B // NCORES          # 32 batch per core
BLK = 16                  # scan steps per pipeline block
COLS = BLK * BS           # 512 columns per block
KH = HID + 2              # state rows + two ones-rows (bias hi/lo)

TRACE = False
LAST_EXEC_NS = None
_CACHE = {}


def _seq(*fs):
    def f(anc):
        for g in fs:
            g(anc)
    return f


def build(t_total=T):
    nblk = t_total // BLK
    nc = bacc.Bacc("TRN2", target_bir_lowering=False, debug=False)

    xT = nc.dram_tensor("xT", [D, t_total * BS], F32, kind="ExternalInput")
    WzT = nc.dram_tensor("WzT", [128, 4, MIX], F32, kind="ExternalInput")
    Wih = nc.dram_tensor("Wih", [MIX + 1, 3 * HID], F32, kind="ExternalInput")
    # bf16 stationaries for the scan: gates r, u, -u, n ([KH, 4*HID]);
    # rows HID:KH are zeros except the n-gate, which carries b_hh_n hi/lo.
    Whh = nc.dram_tensor("Whh", [KH, 4 * HID], F16, kind="ExternalInput")
    I96 = nc.dram_tensor("I96", [HID, HID], F16, kind="ExternalInput")
    WheadT = nc.dram_tensor("WheadT", [HID, MIX], F32, kind="ExternalInput")
    bhead = nc.dram_tensor("bhead", [MIX, 1], F32, kind="ExternalInput")
    Wmix = nc.dram_tensor("Wmix", [MIX, D], F32, kind="ExternalInput")
    yT = nc.dram_tensor("yT", [D, BS], F32, kind="ExternalOutput")

    with tile.TileContext(nc) as tc:
        with (
            tc.tile_pool(name="wts", bufs=1) as wts,
            tc.tile_pool(name="xp", bufs=9) as xp,
            tc.tile_pool(name="zp", bufs=2) as zp,
            tc.tile_pool(name="gbp", bufs=3) as gbp,
            tc.tile_pool(name="gnp", bufs=3) as gnp,
            tc.tile_pool(name="hp", bufs=3) as hp,
            tc.tile_pool(name="gate", bufs=3) as gate,
            tc.tile_pool(name="outp", bufs=2) as outp,
            tc.tile_pool(name="zps", bufs=1, space="PSUM") as zps,
            tc.tile_pool(name="gxps", bufs=3, space="PSUM") as gxps,
            tc.tile_pool(name="ps1", bufs=2, space="PSUM") as ps1p,
            tc.tile_pool(name="ps2", bufs=2, space="PSUM") as ps2p,
        ):
            # ---- persistent weights in SBUF ----
            wz = wts.tile([128, 4, MIX], F32, tag="wz")
            nc.sync.dma_start(wz[:], WzT[:])
            wih = wts.tile([MIX + 1, 3 * HID], F32, tag="wih")
            nc.sync.dma_start(wih[:], Wih[:])
            whh = wts.tile([KH, 4 * HID], F16, tag="whh")
            nc.sync.dma_start(whh[:], Whh[:])
            i96 = wts.tile([HID, HID], F16, tag="i96")
            nc.sync.dma_start(i96[:], I96[:])
            whd = wts.tile([HID, MIX], F32, tag="whd")
            nc.sync.dma_start(whd[:], WheadT[:])
            bhd = wts.tile([MIX, 1], F32, tag="bhd")
            nc.sync.dma_start(bhd[:], bhead[:])
            wmx = wts.tile([MIX, D], F32, tag="wmx")
            nc.sync.dma_start(wmx[:], Wmix[:])

            # ---- ACT table warmup (sigmoid/tanh share one table set) ----
            scr = gate.tile([HID, BS], F32, tag="scr")
            nc.gpsimd.memset(scr[:], 0.0)
            nc.scalar.activation(scr[:], scr[:], AFT.Sigmoid)
            nc.scalar.activation(scr[:], scr[:], AFT.Tanh)

            # ---- initial hidden state ----
            h = hp.tile([KH, BS], F16)
            nc.gpsimd.memset(h[0:HID, :], 0.0)
            nc.gpsimd.memset(h[HID:KH, :], 1.0)

            def dma_block(j):
                xts = []
                for k in range(4):
                    xt = xp.tile([128, COLS], F32)
                    nc.sync.dma_start(
                        xt[:], xT[k * 128:(k + 1) * 128, j * COLS:(j + 1) * COLS]
                    )
                    xts.append(xt)
                return xts

            def make_chunks(j, xts):
                """Precompute block j as a list of small closures, each sized
                to hide inside one scan step's PE/DVE idle window.

                gb[:, i, :] holds fp16 (gxb_r | gxb_u | -gxb_u) for step i;
                gn holds fp32 gx_n (t-major, 32 batch cols per step)."""
                HC = COLS // 2  # 256-column halves
                ztile = zp.tile([MIX + 1, COLS], F32)
                zpsum = zps.tile([MIX, COLS], F32)
                gb = gbp.tile([HID, BLK, 3 * BS], F16)
                gn = gnp.tile([HID, COLS], F32)
                gps_half = {}
                pieces = []

                def _pe(i, anc):
                    if anc and anc[0] is not None:
                        add_dep_helper(i.ins, anc[0].ins, sync=False,
                                       reason="piece after step PE")

                def _dve(i, anc):
                    if anc and anc[1] is not None:
                        add_dep_helper(i.ins, anc[1].ins, sync=False,
                                       reason="piece after step DVE")

                def zmm(k, hh):
                    def f(anc):
                        _pe(nc.tensor.matmul(
                            zpsum[:, hh * HC:(hh + 1) * HC],
                            wz[:, k, :], xts[k][:, hh * HC:(hh + 1) * HC],
                            start=(k == 0), stop=(k == 3),
                        ), anc)
                    return f

                def zcopy(hh):
                    def f(anc):
                        _dve(nc.vector.tensor_copy(
                            ztile[0:MIX, hh * HC:(hh + 1) * HC],
                            zpsum[:, hh * HC:(hh + 1) * HC],
                        ), anc)
                        if hh == 0:
                            nc.gpsimd.memset(ztile[MIX:MIX + 1, :], 1.0)
                    return f

                def gxmm(gi, hh):
                    def f(anc):
                        gps = gxps.tile([HID, HC], F32)
                        gps_half[(gi, hh)] = gps
                        _pe(nc.tensor.matmul(
                            gps[:], wih[:, gi * HID:(gi + 1) * HID],
                            ztile[:, hh * HC:(hh + 1) * HC],
                            start=True, stop=True,
                        ), anc)
                    return f

                def gcopy(gi, hh):
                    def f(anc):
                        gps = gps_half.pop((gi, hh))
                        src = gps[:].rearrange("p (t b) -> p t b", b=BS)
                        trng = slice(hh * (BLK // 2), (hh + 1) * (BLK // 2))
                        if gi < 2:
                            _dve(nc.vector.tensor_copy(
                                gb[:, trng, gi * BS:(gi + 1) * BS], src
                            ), anc)
                            if gi == 1:  # negated copy for the (1-u) column
                                _dve(nc.vector.tensor_scalar(
                                    gb[:, trng, 2 * BS:3 * BS], src,
                                    -1.0, None, op0=OP.mult,
                                ), anc)
                        else:
                            _dve(nc.vector.tensor_copy(
                                gn[:, hh * HC:(hh + 1) * HC], gps[:]
                            ), anc)
                    return f

                for k in range(4):
                    pieces.append(zmm(k, 0))
                for k in range(4):
                    pieces.append(zmm(k, 1))
                pieces[3] = _seq(pieces[3], zcopy(0))
                pieces[7] = _seq(pieces[7], zcopy(1))
                # gx matmuls interleaved with their evacuation copies
                pieces.append(gxmm(0, 0))            # p8
                pieces.append(_seq(gxmm(1, 0), gcopy(0, 0)))   # p9
                pieces.append(_seq(gxmm(2, 0), gcopy(1, 0)))   # p10
                pieces.append(_seq(gxmm(0, 1), gcopy(2, 0)))   # p11
                pieces.append(_seq(gxmm(1, 1), gcopy(0, 1)))   # p12
                pieces.append(_seq(gxmm(2, 1), gcopy(1, 1)))   # p13
                pieces.append(gcopy(2, 1))           # p14
                return gb, gn, pieces

            def imm(gb, i):
                """Inject precomputed gate inputs for step i into a fresh
                PSUM bank (start=True) — issued one step ahead."""
                ps1 = ps1p.tile([HID, 3 * BS], F32, tag="ps1")
                nc.tensor.matmul(ps1[:], i96[:], gb[:, i, :],
                                 start=True, stop=False)
                return ps1

            def scan_step(h, ps1, gn, i):
                nc.tensor.matmul(ps1[:, 0:BS], whh[:, 0:HID], h[:],
                                 start=False, stop=False)
                nc.tensor.matmul(ps1[:, BS:2 * BS], whh[:, HID:2 * HID], h[:],
                                 start=False, stop=False)
                last_mm = nc.tensor.matmul(
                    ps1[:, 2 * BS:3 * BS], whh[:, 2 * HID:3 * HID],
                    h[:], start=False, stop=True)
                ps2 = ps2p.tile([HID, 2 * BS], F32, tag="ps2")
                nc.tensor.matmul(ps2[:, 0:BS], whh[:, 3 * HID:4 * HID], h[:],
                                 start=True, stop=True)

                r = gate.tile([HID, BS], F32, tag="r")
                nc.scalar.activation(r[:], ps1[:, 0:BS], AFT.Sigmoid)
                uu = gate.tile([HID, 2 * BS], F32, tag="uu")
                nc.scalar.activation(uu[:], ps1[:, BS:3 * BS], AFT.Sigmoid)

                tn = gate.tile([HID, BS], F32, tag="tn")
                nc.vector.tensor_tensor(tn[:], ps2[:, 0:BS], r[:], op=OP.mult)
                nc.vector.tensor_tensor(
                    ps2[:, BS:2 * BS], tn[:], gn[:, i * BS:(i + 1) * BS],
                    op=OP.add,
                )
                nn = gate.tile([HID, BS], F32, tag="nn")
                nc.scalar.activation(nn[:], ps2[:, BS:2 * BS], AFT.Tanh)

                uh = gate.tile([HID, BS], F32, tag="uh")
                nc.vector.tensor_tensor(uh[:], uu[:, 0:BS], h[0:HID, :],
                                        op=OP.mult)
                h2 = hp.tile([KH, BS], F16)
                nc.gpsimd.memset(h2[HID:KH, :], 1.0)
                nc.vector.tensor_tensor(h2[0:HID, :], uu[:, BS:2 * BS], nn[:],
                                        op=OP.mult)
                last_dve = nc.vector.tensor_tensor(
                    h2[0:HID, :], h2[0:HID, :], uh[:], op=OP.add)
                return h2, (last_mm, last_dve)

            # ---- pipelined precompute + scan ----
            # x-DMAs issued two blocks ahead of their matmuls; precompute
            # pieces for block j+2 drip one-per-step through block j.
            blocks = {}
            for j in range(min(3, nblk)):
                xts = dma_block(j)
                if j < 2:
                    gbj, gnj, pieces = make_chunks(j, xts)
                    for p in pieces:
                        p(None)
                    blocks[j] = (gbj, gnj, xts)
                else:
                    blocks[j] = (None, None, xts)

            ps1 = imm(blocks[0][0], 0)
            for j in range(nblk):
                if j + 3 < nblk:
                    blocks[j + 3] = (None, None, dma_block(j + 3))
                pend = []
                if j + 2 < nblk:
                    gbj, gnj, pieces = make_chunks(j + 2, blocks[j + 2][2])
                    blocks[j + 2] = (gbj, gnj, None)
                    pend = pieces
                cur_gb, cur_gn = blocks[j][0], blocks[j][1]
                for i in range(BLK):
                    h2, anc = scan_step(h, ps1, cur_gn, i)
                    if i < len(pend):
                        pend[i](anc)
                    # inject next step's gate inputs while this chain runs
                    last = (j == nblk - 1) and (i == BLK - 1)
                    if not last:
                        if i == BLK - 1:
                            ps1 = imm(blocks[j + 1][0], 0)
                        else:
                            ps1 = imm(cur_gb, i + 1)
                    h = h2
                blocks.pop(j)

            # ---- head: z_next = Whead @ h + bhead ; y.T = Wmix.T @ z_next ----
            hf = gate.tile([HID, BS], F32, tag="hf")
            nc.vector.tensor_copy(hf[:], h[0:HID, :])
            znps = ps1p.tile([MIX, BS], F32, tag="ps1")
            nc.tensor.matmul(znps[:], whd[:], hf[:], start=True, stop=True)
            zn = gate.tile([MIX, BS], F32, tag="zn")
            nc.vector.tensor_scalar(zn[:], znps[:], bhd[:], None, op0=OP.add)
            for k in range(4):
                yps = ps2p.tile([128, BS], F32, tag="ps2")
                nc.tensor.matmul(yps[:], wmx[:, k * 128:(k + 1) * 128], zn[:],
                                 start=True, stop=True)
                yt = outp.tile([128, BS], F32)
                nc.vector.tensor_copy(yt[:], yps[:])
                nc.sync.dma_start(yT[k * 128:(k + 1) * 128, :], yt[:])

    nc.compile()
    return nc


def _f16(a):
    return np.asarray(a, np.float32).astype(np.float16)


def prep_weights(W_mix, W_ih, W_hh, b_ih, b_hh, W_head, b_head):
    W_mix = np.asarray(W_mix, np.float32)
    W_ih = np.asarray(W_ih, np.float32)
    W_hh = np.asarray(W_hh, np.float32)
    b_ih = np.asarray(b_ih, np.float32)
    b_hh = np.asarray(b_hh, np.float32)
    W_head = np.asarray(W_head, np.float32)
    b_head = np.asarray(b_head, np.float32)

    # WzT[p, k, m] = W_mix[m, 128k + p]
    WzT = np.ascontiguousarray(
        W_mix.T.reshape(4, 128, MIX).transpose(1, 0, 2)
    )
    # Wih_hat: [MIX+1, 3H]; per gate columns = [W_ih_g.T ; fused bias]
    gates_b = [
        b_ih[0:HID] + b_hh[0:HID],
        b_ih[HID:2 * HID] + b_hh[HID:2 * HID],
        b_ih[2 * HID:3 * HID],
    ]
    Wih_hat = np.zeros((MIX + 1, 3 * HID), np.float32)
    for g in range(3):
        Wih_hat[0:MIX, g * HID:(g + 1) * HID] = W_ih[g * HID:(g + 1) * HID].T
        Wih_hat[MIX, g * HID:(g + 1) * HID] = gates_b[g]

    # bf16 scan stationaries [KH, 4H]: r, u, -u, n; n carries b_hh_n hi/lo.
    Whh_hat = np.zeros((KH, 4 * HID), np.float32)
    Wr, Wu, Wn = (W_hh[g * HID:(g + 1) * HID] for g in range(3))
    Whh_hat[0:HID, 0:HID] = Wr.T
    Whh_hat[0:HID, HID:2 * HID] = Wu.T
    Whh_hat[0:HID, 2 * HID:3 * HID] = -Wu.T
    Whh_hat[0:HID, 3 * HID:4 * HID] = Wn.T
    bn = b_hh[2 * HID:3 * HID]
    bn_hi = bn.astype(np.float16).astype(np.float32)
    Whh_hat[HID, 3 * HID:4 * HID] = bn_hi
    Whh_hat[HID + 1, 3 * HID:4 * HID] = bn - bn_hi
    return {
        "WzT": WzT,
        "Wih": Wih_hat,
        "Whh": _f16(Whh_hat),
        "I96": _f16(np.eye(HID, dtype=np.float32)),
        "WheadT": np.ascontiguousarray(W_head.T),
        "bhead": np.ascontiguousarray(b_head[:, None]),
        "Wmix": W_mix,
    }


def kernel(x, W_mix, W_ih, W_hh, b_ih, b_hh, W_head, b_head):
    global LAST_EXEC_NS
    if "nc" not in _CACHE:
        _CACHE["nc"] = build(T)
    nc = _CACHE["nc"]

    wmap = prep_weights(W_mix, W_ih, W_hh, b_ih, b_hh, W_head, b_head)
    x = np.asarray(x, np.float32)
    in_maps = []
    for c in range(NCORES):
        xc = x[c * BS:(c + 1) * BS]                       # [BS, T, D]
        xTc = np.ascontiguousarray(xc.transpose(2, 1, 0)).reshape(D, T * BS)
        in_maps.append({"xT": xTc, **wmap})

    res = run_bass_kernel_spmd(
        nc, in_maps, core_ids=list(range(NCORES)), trace=TRACE
    )
    LAST_EXEC_NS = res.exec_time_ns
    y = np.empty((B, D), np.float32)
    for c in range(NCORES):
        y[c * BS:(c + 1) * BS] = res.results[c]["yT"].T
    return y
